# revision 41
# baseline (speedup 1.0000x reference)
"""MLA (CustomLlamaMLAForInfer) Trainium2 Bass kernel, v3.

Sharding: tensor-parallel over heads across 8 NeuronCores. Core c owns
kv-head c and q-heads [4c, 4c+4). Every core sees the full token stream
(B*S = 4096 tokens). The shared low-rank latent (c_kv, 512 dims) is
*sharded*: core c computes latent dims [64c, 64c+64) for all tokens and
an AllGather rebuilds the full latent on every core. o_proj is
row-sharded; the host sums the 8 partial [4096, 4096] outputs.

All matmuls in bf16 (inputs pre-converted host-side), fp32 PSUM.
One streaming TileContext; PE executes strictly in emission order:

  A(g), g=0..7 (512-token blocks): qT = Wq_shard @ hid.T (rope folded
     in at evict, kept in SBUF), [c_kv shard; k_rope shard] fused
     matmul; c_kv shard -> DRAM -> AllGather (overlapped with later
     A blocks); k_rope roped+scattered into SBUF kT.
  B(g): k_nope/v of the core's kv head from the gathered latent.
  C(g): causal attention for q-block g, 4 q-heads. k-tiles processed
     in PAIRS: two 512-col score matmuls into one 2-bank [128,1024]
     PSUM tile, ONE exp (ScalarE) per pair, paired causal masks,
     v.T@p + ones-matmul denominators, one-pair software pipeline.
  D(g): partial o_proj; PSUM evicted straight to DRAM via DMA (f32).

PSUM (8 banks): big [128,1024] x2 (A q-pairs / C score-pairs),
mid [128,512] x2 (A dk+kr / B knope,v / C out-accum / D o_proj),
sum [1,512] x2 (softmax denominators).
"""

import numpy as np

HIDDEN = 4096
N_HEADS = 32
KV_HEADS = 8
HEAD_DIM = 128
LOW_RANK = 64
TOP_K_ROPE = 32
ROPE_THETA = 10000.0
B, S = 2, 2048
NCORES = 8
HPC = N_HEADS // NCORES          # q heads per core = 4
QR = HPC * HEAD_DIM              # q rows per core = 512
CD = LOW_RANK * KV_HEADS         # latent dim = 512
LSH = CD // NCORES               # latent shard per core = 64
KRR = 2 * TOP_K_ROPE             # rope rows per kv head = 64
NT = B * S                       # total tokens = 4096
TB = 512                         # token block
NG = NT // TB                    # token blocks = 8
HT = HIDDEN // 128               # hidden k-tiles = 32
LT = CD // 128                   # latent k-tiles = 4
NJ = TB // 128                   # diag mask variants = 4


def _rope_tables(seq_len):
    inv = 1.0 / (ROPE_THETA ** (np.arange(0, HEAD_DIM, 2, dtype=np.float32) / HEAD_DIM))
    pos = np.arange(seq_len, dtype=np.float32)
    fr = np.outer(pos, inv)
    emb = np.concatenate([fr, fr], axis=-1)          # [S, 128]
    return (np.cos(emb).T.astype(np.float32),        # [128, S]
            np.sin(emb).T.astype(np.float32))


def build_program(trace_sim=False):
    from concourse import bacc, tile, mybir
    import concourse.bass as bass

    f32 = mybir.dt.float32
    F32R = mybir.dt.float32r
    bf16 = mybir.dt.bfloat16
    MS = bass.MemorySpace
    EXP = mybir.ActivationFunctionType.Exp

    nc = bacc.Bacc("TRN2", target_bir_lowering=False, debug=False,
                   num_devices=NCORES)

    def din(name, shape):
        return nc.dram_tensor(name, shape, bf16, kind="ExternalInput").ap()

    hidT = din("hidT", [HIDDEN, NT])
    wq = din("wq_t", [HIDDEN, QR])
    wdkkr = din("wdkkr_t", [HIDDEN, 128])
    wupk = din("wupk_t", [CD, KRR])
    wupv = din("wupv_t", [CD, HEAD_DIM])
    wo = din("wo_t", [QR, HIDDEN])
    qcos = din("qcos", [128, S])
    qsin = din("qsin", [128, S])
    kcos = din("kcos", [128, S])     # rows 64:128 hold the values
    ksin = din("ksin", [128, S])     # rows 64:128 hold the values
    masks2 = din("masks2", [128, NJ // 2, 2 * TB])
    outp = nc.dram_tensor("out_part", [NT, HIDDEN], bf16,
                          kind="ExternalOutput").ap()

    rg = [list(range(NCORES))]

    with tile.TileContext(nc, trace_sim=trace_sim) as tc:
        with tc.tile_pool(name="pers", bufs=1) as pers, \
             tc.tile_pool(name="ring", bufs=2) as ring, \
             tc.tile_pool(name="dram", bufs=8, space="DRAM") as dram, \
             tc.tile_pool(name="ps", bufs=1, space=MS.PSUM) as psp:

            # ---------------- persistent SBUF ----------------
            # A-phase-critical loads first; wq split so A(0) starts after
            # its first chunk lands.
            wq_sb = pers.tile([128, HT, QR], bf16, tag="wq")
            wqr = wq.rearrange("(t p) w -> p t w", p=128)
            for ch in range(4):
                t0, t1 = ch * (HT // 4), (ch + 1) * (HT // 4)
                nc.sync.dma_start(wq_sb[:, t0:t1, :], wqr[:, t0:t1, :])
            wdkkr_sb = pers.tile([128, HT, 128], bf16, tag="wdkkr")
            nc.sync.dma_start(wdkkr_sb[:], wdkkr.rearrange("(t p) w -> p t w", p=128))
            qcos_sb = pers.tile([128, S], bf16, tag="qcos")
            nc.sync.dma_start(qcos_sb[:], qcos)
            qsin_sb = pers.tile([128, S], bf16, tag="qsin")
            nc.sync.dma_start(qsin_sb[:], qsin)
            kcos_sb = pers.tile([128, S], bf16, tag="kcos")
            nc.sync.dma_start(kcos_sb[:], kcos)
            ksin_sb = pers.tile([128, S], bf16, tag="ksin")
            nc.sync.dma_start(ksin_sb[:], ksin)
            # BCD-phase tiles (DMAs emitted after phase A so they don't
            # contend with the A-critical hid/wq loads)
            wupk_sb = pers.tile([128, LT, KRR], bf16, tag="wupk")
            wupv_sb = pers.tile([128, LT, HEAD_DIM], bf16, tag="wupv")
            wo_sb = pers.tile([128, HPC, HIDDEN], bf16, tag="wo")
            masks_sb = pers.tile([128, NJ // 2, 2 * TB], bf16, tag="masks")
            ones_f32 = pers.tile([128, 1], f32, tag="ones_f32")
            nc.vector.memset(ones_f32[:], 1.0)
            ones32_sb = pers.tile([128, 1], F32R, tag="ones32")
            nc.scalar.copy(ones32_sb[:], ones_f32[:])

            for z in range(3):
                ptz = ring.tile([128, 2 * TB], bf16, tag="pt2", bufs=4,
                                name=f"ptz{z}")
                nc.vector.memset(ptz[:], 0.0)

            qT_sb = pers.tile([128, HPC, NT], bf16, tag="qT")
            kT_sb = pers.tile([128, NT], bf16, tag="kT")
            v_sb = pers.tile([128, NT // 128, HEAD_DIM], bf16, tag="v")

            gaths = []

            # ================ phase A: projections ================
            for g in range(NG):
                c0, c1 = g * TB, (g + 1) * TB
                p0 = (g % (S // TB)) * TB          # table col (per batch)
                p1 = p0 + TB
                qp2 = [psp.tile([128, 2 * TB], f32, tag="big", bufs=2,
                                name=f"qp2_{g}_{i}") for i in range(2)]
                dkp = psp.tile([128, TB], f32, tag="mid", bufs=2,
                               name=f"dkp{g}")[:]
                for t in range(HT):
                    ht = ring.tile([128, TB], bf16, tag="hid", bufs=8,
                                   name=f"hid{g}_{t}")
                    nc.sync.dma_start(ht[:], hidT[t * 128:(t + 1) * 128, c0:c1])
                    for m in range(HPC):
                        nc.tensor.matmul(
                            qp2[m // 2][:, (m % 2) * TB:(m % 2 + 1) * TB],
                            wq_sb[:, t, m * 128:(m + 1) * 128],
                            ht[:],
                            start=(t == 0), stop=(t == HT - 1))
                    nc.tensor.matmul(dkp, wdkkr_sb[:, t, :], ht[:],
                                     start=(t == 0), stop=(t == HT - 1))
                # ---- evict q heads with rope ----
                for i in range(2):
                    raw2 = ring.tile([128, 2 * TB], bf16, tag="raw",
                                     name=f"raw{g}_{i}")
                    nc.scalar.copy(raw2[:], qp2[i][:])
                    for mh in range(2):
                        m = 2 * i + mh
                        raw = raw2[:, mh * TB:(mh + 1) * TB]
                        rot = ring.tile([128, TB], bf16, tag="rot",
                                        name=f"rot{g}_{m}")
                        nc.sync.dma_start(rot[0:64, :], raw[64:128, :])
                        nc.sync.dma_start(rot[64:128, :], raw[0:64, :])
                        qsb = ring.tile([128, TB], bf16, tag="qsb",
                                        name=f"qsb{g}_{m}")
                        nc.vector.tensor_mul(qsb[:], raw[:], qcos_sb[:, p0:p1])
                        nc.vector.tensor_mul(rot[:], rot[:], qsin_sb[:, p0:p1])
                        nc.vector.tensor_add(qT_sb[:, m, c0:c1], qsb[:], rot[:])
                # ---- evict latent shard + k rope ----
                dka = ring.tile([128, TB], bf16, tag="dka", name=f"dka{g}")
                nc.scalar.copy(dka[:], dkp)
                ckv_my = dram.tile([LSH, TB], bf16, tag="ckv_my", name=f"ckvmy{g}")
                nc.sync.dma_start(ckv_my[:], dka[0:LSH, :])
                gath = dram.tile([CD, TB], bf16, tag="gath", addr_space="Shared",
                                 name=f"gath{g}")
                nc.gpsimd.collective_compute(
                    "AllGather", mybir.AluOpType.bypass, replica_groups=rg,
                    ins=[ckv_my.opt()], outs=[gath.opt()])
                gaths.append(gath)
                # rope rows live at partitions 64:128
                rotk = ring.tile([128, TB], bf16, tag="rotk", name=f"rotk{g}")
                nc.sync.dma_start(rotk[64:96, :], dka[96:128, :])
                nc.sync.dma_start(rotk[96:128, :], dka[64:96, :])
                ktm = ring.tile([128, TB], bf16, tag="ktm", name=f"ktm{g}")
                nc.vector.tensor_mul(ktm[64:128, :], dka[64:128, :],
                                     kcos_sb[64:128, p0:p1])
                nc.vector.tensor_mul(rotk[64:128, :], rotk[64:128, :],
                                     ksin_sb[64:128, p0:p1])
                nc.vector.tensor_add(ktm[64:128, :], ktm[64:128, :],
                                     rotk[64:128, :])
                nc.sync.dma_start(kT_sb[0:32, c0:c1], ktm[64:96, :])
                nc.sync.dma_start(kT_sb[64:96, c0:c1], ktm[96:128, :])

            # deferred BCD weight loads (complete during remaining A blocks)
            nc.sync.dma_start(wupk_sb[:], wupk.rearrange("(t p) w -> p t w", p=128))
            nc.sync.dma_start(wupv_sb[:], wupv.rearrange("(t p) w -> p t w", p=128))
            nc.sync.dma_start(masks_sb[:], masks2)
            nc.sync.dma_start(wo_sb[:], wo.rearrange("(h p) w -> p h w", p=128))

            # ================ phases B/C/D per block ================
            def emit_B(g):
                c0, c1 = g * TB, (g + 1) * TB
                cb = ring.tile([128, LT, TB], bf16, tag="cb", name=f"cb{g}")
                nc.sync.dma_start(cb[:], gaths[g].rearrange("(l p) w -> p l w", p=128))
                kcp = psp.tile([KRR, TB], f32, tag="mid", bufs=2,
                               name=f"kcp{g}")[:]
                for lt in range(LT):
                    nc.tensor.matmul(kcp, wupk_sb[:, lt, :], cb[:, lt, :],
                                     start=(lt == 0), stop=(lt == LT - 1))
                kcs = ring.tile([KRR, TB], bf16, tag="kcs", name=f"kcs{g}")
                nc.vector.tensor_copy(kcs[:], kcp)
                nc.sync.dma_start(kT_sb[32:64, c0:c1], kcs[0:32, :])
                nc.sync.dma_start(kT_sb[96:128, c0:c1], kcs[32:64, :])
                for tt in range(TB // 128):
                    vp = psp.tile([128, HEAD_DIM], f32, tag="mid", bufs=2,
                                  name=f"vp{g}_{tt}")[:]
                    for lt in range(LT):
                        nc.tensor.matmul(vp,
                                         cb[:, lt, tt * 128:(tt + 1) * 128],
                                         wupv_sb[:, lt, :],
                                         start=(lt == 0), stop=(lt == LT - 1))
                    nc.vector.tensor_copy(v_sb[:, g * (TB // 128) + tt, :], vp)

            emit_B(0)
            for g in range(NG):
                c0, c1 = g * TB, (g + 1) * TB
                b, qb = g // (S // TB), g % (S // TB)
                off = b * S
                # ---- C: attention for q-block (b, qb), 4 heads ----
                atn = ring.tile([128, HPC, TB], bf16, tag="atn", name=f"atn{g}")
                nkt = (qb + 1) * NJ
                npair = nkt // 2
                for h in range(HPC):
                    qsl = qT_sb[:, h, off + qb * TB: off + (qb + 1) * TB]
                    ops = psp.tile([128, TB], f32, tag="mid", bufs=2,
                                   name=f"ops{g}_{h}")[:]
                    acc = ring.tile([128, 2 * TB], F32R, tag="acc", bufs=2,
                                    name=f"acc{g}_{h}")
                    pts = [None, None, None, None]
                    # q0s[kt] = first causally-active q column of k-tile kt;
                    # only the last diagonal pair is restricted (the first
                    # one is nearly full anyway).
                    def q0_of(kt):
                        j = kt - qb * NJ
                        return j * 128 if j >= 1 else 0

                    for p in range(npair + 3):
                        if p < npair:
                            sc2 = psp.tile([128, 2 * TB], f32, tag="big", bufs=2,
                                           name=f"sc2_{g}_{h}_{p}")
                            for u in range(2):
                                kt = 2 * p + u
                                q0 = q0_of(kt)
                                nc.tensor.matmul(
                                    sc2[:, u * TB + q0:(u + 1) * TB],
                                    kT_sb[:, off + kt * 128: off + (kt + 1) * 128],
                                    qsl[:, q0:],
                                    start=True, stop=True)
                            pt2 = ring.tile([128, 2 * TB], bf16, tag="pt2", bufs=4,
                                            name=f"pt2_{g}_{h}_{p}")
                            if 2 * p - qb * NJ >= 2:
                                # last diagonal pair: exp only the causally
                                # valid tail (mask zeroes the stale rest)
                                for u in range(2):
                                    q0 = q0_of(2 * p + u)
                                    nc.scalar.activation(
                                        pt2[:, u * TB + q0:(u + 1) * TB],
                                        sc2[:, u * TB + q0:(u + 1) * TB], EXP)
                            else:
                                nc.scalar.activation(pt2[:], sc2[:], EXP)
                            jp = p - qb * (NJ // 2)
                            if jp >= 0:
                                # full-width: also zeroes stale data left of q0
                                nc.vector.tensor_mul(pt2[:], pt2[:],
                                                     masks_sb[:, jp, :])
                            pts[p % 4] = pt2
                        pp = p - 3
                        if pp >= 0:
                            ppt = pts[pp % 4]
                            for u in range(2):
                                kt = 2 * pp + u
                                q0 = q0_of(kt)
                                nc.tensor.matmul(
                                    ops[:, q0:TB],
                                    v_sb[:, b * (S // 128) + kt, :],
                                    ppt[:, u * TB + q0:(u + 1) * TB],
                                    start=(kt == 0), stop=(kt == nkt - 1))
                            # denominator partial sums on DVE, emitted after
                            # the AV matmuls so they don't delay the next
                            # pair's mask-mul in the DVE stream
                            if pp == 0:
                                nc.vector.tensor_copy(acc[:], ppt[:])
                            else:
                                nc.vector.tensor_add(acc[:], acc[:], ppt[:])
                    # denominators: reduce acc halves with a ones-matmul
                    sps = psp.tile([1, TB], f32, tag="sum", bufs=2,
                                   name=f"sps{g}_{h}")[:]
                    for u in range(2):
                        nc.tensor.matmul(sps, ones32_sb[:],
                                         acc[:, u * TB:(u + 1) * TB],
                                         start=(u == 0), stop=(u == 1))
                    rec = ring.tile([1, TB], f32, tag="rec", bufs=1,
                                    name=f"rec{g}_{h}")
                    nc.vector.reciprocal_approx_fast(rec[:], sps)
                    rbs = ring.tile([128, TB], f32, tag="rbs", name=f"rbs{g}_{h}")
                    nc.gpsimd.partition_broadcast(rbs[:], rec[:])
                    nc.vector.tensor_mul(atn[:, h, :], ops, rbs[:])

                # B of the next block overlaps the normalize tail of C(g)
                if g + 1 < NG:
                    emit_B(g + 1)

                # ---- D: partial o_proj (1024-wide moving) ----
                for T in range(TB // 128):
                    for n in range(HIDDEN // 1024):
                        ps2 = psp.tile([128, 2 * TB], f32, tag="big", bufs=2,
                                       name=f"od_{g}_{T}_{n}")
                        for half in range(2):
                            for h2 in range(HPC):
                                nc.tensor.matmul(
                                    ps2[:, half * TB:(half + 1) * TB],
                                    atn[:, h2, T * 128:(T + 1) * 128],
                                    wo_sb[:, h2, n * 1024 + half * TB:
                                          n * 1024 + (half + 1) * TB],
                                    start=(h2 == 0), stop=(h2 == HPC - 1))
                        osb = ring.tile([128, 2 * TB], bf16, tag="osb", bufs=2,
                                        name=f"osb{g}_{T}_{n}")
                        nc.vector.tensor_copy(osb[:], ps2[:])
                        nc.sync.dma_start(
                            outp[c0 + T * 128: c0 + (T + 1) * 128,
                                 n * 1024:(n + 1) * 1024], osb[:])

    nc.compile()
    return nc


def make_in_maps(hidden_states, Wq, Wkr, Wdk, Wupk, Wupv, Wo):
    """Host-side sharding + layout prep (bf16). Returns per-core input dicts."""
    import ml_dtypes
    bf16 = ml_dtypes.bfloat16
    scale = 1.0 / np.sqrt(np.float32(HEAD_DIM))

    hidT = np.ascontiguousarray(
        np.asarray(hidden_states, np.float32).reshape(NT, HIDDEN).T).astype(bf16)

    cos_t, sin_t = _rope_tables(S)                     # [128, S] f32
    qcos = (cos_t * scale).astype(bf16)
    qsin = (np.concatenate([-sin_t[0:64], sin_t[64:128]], axis=0) * scale).astype(bf16)
    kcos = np.zeros((128, S), np.float32)
    ksin = np.zeros((128, S), np.float32)
    kcos[64:96] = cos_t[0:32]
    kcos[96:128] = cos_t[64:96]
    ksin[64:96] = -sin_t[0:32]
    ksin[96:128] = sin_t[64:96]
    kcos = kcos.astype(bf16)
    ksin = ksin.astype(bf16)

    k_idx = np.arange(128)[:, None]
    q_idx = np.arange(TB)[None, :]
    m1 = np.stack(
        [(q_idx >= j * 128 + k_idx).astype(np.float32) for j in range(NJ)],
        axis=1)                                        # [128, NJ, TB]
    masks2 = np.concatenate(
        [np.stack([m1[:, 0], m1[:, 2]], axis=1),
         np.stack([m1[:, 1], m1[:, 3]], axis=1)], axis=2).astype(bf16)
    # masks2[:, i, 0:TB] = mask_{2i}, masks2[:, i, TB:2TB] = mask_{2i+1}

    Wq = np.asarray(Wq, np.float32)
    Wkr = np.asarray(Wkr, np.float32)
    Wdk = np.asarray(Wdk, np.float32)
    Wupk = np.asarray(Wupk, np.float32)
    Wupv = np.asarray(Wupv, np.float32)
    Wo = np.asarray(Wo, np.float32)

    in_maps = []
    for c in range(NCORES):
        wq_t = np.ascontiguousarray(Wq[QR * c:QR * (c + 1)].T).astype(bf16)
        wdkkr_t = np.ascontiguousarray(
            np.concatenate([Wdk[LSH * c:LSH * (c + 1)],
                            Wkr[KRR * c:KRR * (c + 1)]], axis=0).T).astype(bf16)
        wupk_t = np.ascontiguousarray(Wupk[KRR * c:KRR * (c + 1)].T).astype(bf16)
        wupv_t = np.ascontiguousarray(
            Wupv[HEAD_DIM * c:HEAD_DIM * (c + 1)].T).astype(bf16)
        wo_t = np.ascontiguousarray(Wo[:, QR * c:QR * (c + 1)].T).astype(bf16)
        in_maps.append({
            "hidT": hidT, "wq_t": wq_t, "wdkkr_t": wdkkr_t,
            "wupk_t": wupk_t, "wupv_t": wupv_t, "wo_t": wo_t,
            "qcos": qcos, "qsin": qsin, "kcos": kcos, "ksin": ksin,
            "masks2": masks2,
        })
    return in_maps


_NC_CACHE = {}


def _get_program(key=0):
    if key not in _NC_CACHE:
        _NC_CACHE[key] = build_program()
    return _NC_CACHE[key]


def finish_output(res):
    out = res.results[0]["out_part"].astype(np.float32)
    for i in range(1, NCORES):
        out = out + res.results[i]["out_part"].astype(np.float32)
    return out.reshape(B, S, HIDDEN).astype(np.float32)


def kernel(hidden_states, Wq, Wkr, Wdk, Wupk, Wupv, Wo):
    from concourse.bass_utils import run_bass_kernel_spmd

    in_maps = make_in_maps(hidden_states, Wq, Wkr, Wdk, Wupk, Wupv, Wo)
    nc = _get_program()
    res = run_bass_kernel_spmd(nc, in_maps, list(range(NCORES)))
    return finish_output(res)


# revision 42
# speedup vs baseline: 1.0244x; 1.0244x over previous
"""MLA (CustomLlamaMLAForInfer) Trainium2 Bass kernel, v3.

Sharding: tensor-parallel over heads across 8 NeuronCores. Core c owns
kv-head c and q-heads [4c, 4c+4). Every core sees the full token stream
(B*S = 4096 tokens). The shared low-rank latent (c_kv, 512 dims) is
*sharded*: core c computes latent dims [64c, 64c+64) for all tokens and
an AllGather rebuilds the full latent on every core. o_proj is
row-sharded; the host sums the 8 partial [4096, 4096] outputs.

All matmuls in bf16 (inputs pre-converted host-side), fp32 PSUM.
One streaming TileContext; PE executes strictly in emission order:

  A(g), g=0..7 (512-token blocks): qT = Wq_shard @ hid.T (rope folded
     in at evict, kept in SBUF), [c_kv shard; k_rope shard] fused
     matmul; c_kv shard -> DRAM -> AllGather (overlapped with later
     A blocks); k_rope roped+scattered into SBUF kT.
  B(g): k_nope/v of the core's kv head from the gathered latent.
  C(g): causal attention for q-block g, 4 q-heads. k-tiles processed
     in PAIRS: two 512-col score matmuls into one 2-bank [128,1024]
     PSUM tile, ONE exp (ScalarE) per pair, paired causal masks,
     v.T@p + ones-matmul denominators, one-pair software pipeline.
  D(g): partial o_proj; PSUM evicted straight to DRAM via DMA (f32).

PSUM (8 banks): big [128,1024] x2 (A q-pairs / C score-pairs),
mid [128,512] x2 (A dk+kr / B knope,v / C out-accum / D o_proj),
sum [1,512] x2 (softmax denominators).
"""

import numpy as np

HIDDEN = 4096
N_HEADS = 32
KV_HEADS = 8
HEAD_DIM = 128
LOW_RANK = 64
TOP_K_ROPE = 32
ROPE_THETA = 10000.0
B, S = 2, 2048
NCORES = 8
HPC = N_HEADS // NCORES          # q heads per core = 4
QR = HPC * HEAD_DIM              # q rows per core = 512
CD = LOW_RANK * KV_HEADS         # latent dim = 512
LSH = CD // NCORES               # latent shard per core = 64
KRR = 2 * TOP_K_ROPE             # rope rows per kv head = 64
NT = B * S                       # total tokens = 4096
TB = 512                         # token block
NG = NT // TB                    # token blocks = 8
HT = HIDDEN // 128               # hidden k-tiles = 32
LT = CD // 128                   # latent k-tiles = 4
NJ = TB // 128                   # diag mask variants = 4


def _rope_tables(seq_len):
    inv = 1.0 / (ROPE_THETA ** (np.arange(0, HEAD_DIM, 2, dtype=np.float32) / HEAD_DIM))
    pos = np.arange(seq_len, dtype=np.float32)
    fr = np.outer(pos, inv)
    emb = np.concatenate([fr, fr], axis=-1)          # [S, 128]
    return (np.cos(emb).T.astype(np.float32),        # [128, S]
            np.sin(emb).T.astype(np.float32))


def build_program(trace_sim=False):
    from concourse import bacc, tile, mybir
    import concourse.bass as bass

    f32 = mybir.dt.float32
    F32R = mybir.dt.float32r
    bf16 = mybir.dt.bfloat16
    MS = bass.MemorySpace
    EXP = mybir.ActivationFunctionType.Exp

    nc = bacc.Bacc("TRN2", target_bir_lowering=False, debug=False,
                   num_devices=NCORES)

    def din(name, shape):
        return nc.dram_tensor(name, shape, bf16, kind="ExternalInput").ap()

    hidT = din("hidT", [HIDDEN, NT])
    wq = din("wq_t", [HIDDEN, QR])
    wdkkr = din("wdkkr_t", [HIDDEN, 128])
    wupk = din("wupk_t", [CD, KRR])
    wupv = din("wupv_t", [CD, HEAD_DIM])
    wo = din("wo_t", [QR, HIDDEN])
    qcos = din("qcos", [128, S])
    qsin = din("qsin", [128, S])
    kcos = din("kcos", [128, S])     # rows 64:128 hold the values
    ksin = din("ksin", [128, S])     # rows 64:128 hold the values
    masks2 = din("masks2", [128, NJ // 2, 2 * TB])
    outp = nc.dram_tensor("out_part", [NT, HIDDEN], bf16,
                          kind="ExternalOutput").ap()

    rg = [list(range(NCORES))]

    with tile.TileContext(nc, trace_sim=trace_sim) as tc:
        with tc.tile_pool(name="pers", bufs=1) as pers, \
             tc.tile_pool(name="ring", bufs=2) as ring, \
             tc.tile_pool(name="dram", bufs=8, space="DRAM") as dram, \
             tc.tile_pool(name="ps", bufs=1, space=MS.PSUM) as psp:

            # ---------------- persistent SBUF ----------------
            # A-phase-critical loads first; wq split so A(0) starts after
            # its first chunk lands.
            wq_sb = pers.tile([128, HT, QR], bf16, tag="wq")
            wqr = wq.rearrange("(t p) w -> p t w", p=128)
            for ch in range(4):
                t0, t1 = ch * (HT // 4), (ch + 1) * (HT // 4)
                nc.sync.dma_start(wq_sb[:, t0:t1, :], wqr[:, t0:t1, :])
            wdkkr_sb = pers.tile([128, HT, 128], bf16, tag="wdkkr")
            nc.sync.dma_start(wdkkr_sb[:], wdkkr.rearrange("(t p) w -> p t w", p=128))
            qcos_sb = pers.tile([128, S], bf16, tag="qcos")
            nc.sync.dma_start(qcos_sb[:], qcos)
            qsin_sb = pers.tile([128, S], bf16, tag="qsin")
            nc.sync.dma_start(qsin_sb[:], qsin)
            kcos_sb = pers.tile([128, S], bf16, tag="kcos")
            nc.sync.dma_start(kcos_sb[:], kcos)
            ksin_sb = pers.tile([128, S], bf16, tag="ksin")
            nc.sync.dma_start(ksin_sb[:], ksin)
            # BCD-phase tiles (DMAs emitted after phase A so they don't
            # contend with the A-critical hid/wq loads)
            wupk_sb = pers.tile([128, LT, KRR], bf16, tag="wupk")
            wupv_sb = pers.tile([128, LT, HEAD_DIM], bf16, tag="wupv")
            wo_sb = pers.tile([128, HPC, HIDDEN], bf16, tag="wo")
            masks_sb = pers.tile([128, NJ // 2, 2 * TB], bf16, tag="masks")
            ones_f32 = pers.tile([128, 1], f32, tag="ones_f32")
            nc.vector.memset(ones_f32[:], 1.0)
            ones32_sb = pers.tile([128, 1], F32R, tag="ones32")
            nc.scalar.copy(ones32_sb[:], ones_f32[:])

            for z in range(3):
                ptz = ring.tile([128, 2 * TB], bf16, tag="pt2", bufs=3,
                                name=f"ptz{z}")
                nc.vector.memset(ptz[:], 0.0)

            qT_sb = pers.tile([128, HPC, NT], bf16, tag="qT")
            kT_sb = pers.tile([128, NT], bf16, tag="kT")
            v_sb = pers.tile([128, NT // 128, HEAD_DIM], bf16, tag="v")

            gaths = []

            # ================ phase A: projections ================
            for g in range(NG):
                c0, c1 = g * TB, (g + 1) * TB
                p0 = (g % (S // TB)) * TB          # table col (per batch)
                p1 = p0 + TB
                qp2 = [psp.tile([128, 2 * TB], f32, tag="big", bufs=2,
                                name=f"qp2_{g}_{i}") for i in range(2)]
                dkp = psp.tile([128, TB], f32, tag="mid", bufs=2,
                               name=f"dkp{g}")[:]
                for t in range(HT):
                    ht = ring.tile([128, TB], bf16, tag="hid", bufs=10,
                                   name=f"hid{g}_{t}")
                    nc.sync.dma_start(ht[:], hidT[t * 128:(t + 1) * 128, c0:c1])
                    for m in range(HPC):
                        nc.tensor.matmul(
                            qp2[m // 2][:, (m % 2) * TB:(m % 2 + 1) * TB],
                            wq_sb[:, t, m * 128:(m + 1) * 128],
                            ht[:],
                            start=(t == 0), stop=(t == HT - 1))
                    nc.tensor.matmul(dkp, wdkkr_sb[:, t, :], ht[:],
                                     start=(t == 0), stop=(t == HT - 1))
                # ---- evict q heads with rope ----
                for i in range(2):
                    raw2 = ring.tile([128, 2 * TB], bf16, tag="raw",
                                     name=f"raw{g}_{i}")
                    nc.scalar.copy(raw2[:], qp2[i][:])
                    for mh in range(2):
                        m = 2 * i + mh
                        raw = raw2[:, mh * TB:(mh + 1) * TB]
                        rot = ring.tile([128, TB], bf16, tag="rot",
                                        name=f"rot{g}_{m}")
                        nc.sync.dma_start(rot[0:64, :], raw[64:128, :])
                        nc.sync.dma_start(rot[64:128, :], raw[0:64, :])
                        qsb = ring.tile([128, TB], bf16, tag="qsb",
                                        name=f"qsb{g}_{m}")
                        nc.vector.tensor_mul(qsb[:], raw[:], qcos_sb[:, p0:p1])
                        nc.vector.tensor_mul(rot[:], rot[:], qsin_sb[:, p0:p1])
                        nc.vector.tensor_add(qT_sb[:, m, c0:c1], qsb[:], rot[:])
                # ---- evict latent shard + k rope ----
                dka = ring.tile([128, TB], bf16, tag="dka", name=f"dka{g}")
                nc.scalar.copy(dka[:], dkp)
                ckv_my = dram.tile([LSH, TB], bf16, tag="ckv_my", name=f"ckvmy{g}")
                nc.sync.dma_start(ckv_my[:], dka[0:LSH, :])
                gath = dram.tile([CD, TB], bf16, tag="gath", addr_space="Shared",
                                 name=f"gath{g}")
                nc.gpsimd.collective_compute(
                    "AllGather", mybir.AluOpType.bypass, replica_groups=rg,
                    ins=[ckv_my.opt()], outs=[gath.opt()])
                gaths.append(gath)
                # rope rows live at partitions 64:128
                rotk = ring.tile([128, TB], bf16, tag="rotk", name=f"rotk{g}")
                nc.sync.dma_start(rotk[64:96, :], dka[96:128, :])
                nc.sync.dma_start(rotk[96:128, :], dka[64:96, :])
                ktm = ring.tile([128, TB], bf16, tag="ktm", name=f"ktm{g}")
                nc.vector.tensor_mul(ktm[64:128, :], dka[64:128, :],
                                     kcos_sb[64:128, p0:p1])
                nc.vector.tensor_mul(rotk[64:128, :], rotk[64:128, :],
                                     ksin_sb[64:128, p0:p1])
                nc.vector.tensor_add(ktm[64:128, :], ktm[64:128, :],
                                     rotk[64:128, :])
                nc.sync.dma_start(kT_sb[0:32, c0:c1], ktm[64:96, :])
                nc.sync.dma_start(kT_sb[64:96, c0:c1], ktm[96:128, :])

            # deferred BCD weight loads (complete during remaining A blocks)
            nc.sync.dma_start(wupk_sb[:], wupk.rearrange("(t p) w -> p t w", p=128))
            nc.sync.dma_start(wupv_sb[:], wupv.rearrange("(t p) w -> p t w", p=128))
            nc.sync.dma_start(masks_sb[:], masks2)
            nc.sync.dma_start(wo_sb[:], wo.rearrange("(h p) w -> p h w", p=128))

            # ================ phases B/C/D per block ================
            def emit_B(g):
                c0, c1 = g * TB, (g + 1) * TB
                cb = ring.tile([128, LT, TB], bf16, tag="cb", name=f"cb{g}")
                nc.sync.dma_start(cb[:], gaths[g].rearrange("(l p) w -> p l w", p=128))
                kcp = psp.tile([KRR, TB], f32, tag="mid", bufs=2,
                               name=f"kcp{g}")[:]
                for lt in range(LT):
                    nc.tensor.matmul(kcp, wupk_sb[:, lt, :], cb[:, lt, :],
                                     start=(lt == 0), stop=(lt == LT - 1))
                kcs = ring.tile([KRR, TB], bf16, tag="kcs", name=f"kcs{g}")
                nc.vector.tensor_copy(kcs[:], kcp)
                nc.sync.dma_start(kT_sb[32:64, c0:c1], kcs[0:32, :])
                nc.sync.dma_start(kT_sb[96:128, c0:c1], kcs[32:64, :])
                for tt in range(TB // 128):
                    vp = psp.tile([128, HEAD_DIM], f32, tag="mid", bufs=2,
                                  name=f"vp{g}_{tt}")[:]
                    for lt in range(LT):
                        nc.tensor.matmul(vp,
                                         cb[:, lt, tt * 128:(tt + 1) * 128],
                                         wupv_sb[:, lt, :],
                                         start=(lt == 0), stop=(lt == LT - 1))
                    nc.vector.tensor_copy(v_sb[:, g * (TB // 128) + tt, :], vp)

            emit_B(0)
            for g in range(NG):
                c0, c1 = g * TB, (g + 1) * TB
                b, qb = g // (S // TB), g % (S // TB)
                off = b * S
                # ---- C: attention for q-block (b, qb), 4 heads ----
                atn = ring.tile([128, HPC, TB], bf16, tag="atn", name=f"atn{g}")
                nkt = (qb + 1) * NJ
                npair = nkt // 2
                for h in range(HPC):
                    qsl = qT_sb[:, h, off + qb * TB: off + (qb + 1) * TB]
                    ops = psp.tile([128, TB], f32, tag="mid", bufs=2,
                                   name=f"ops{g}_{h}")[:]
                    acc = ring.tile([128, 2 * TB], F32R, tag="acc", bufs=2,
                                    name=f"acc{g}_{h}")
                    pts = [None, None, None, None]
                    # q0s[kt] = first causally-active q column of k-tile kt;
                    # only the last diagonal pair is restricted (the first
                    # one is nearly full anyway).
                    def q0_of(kt):
                        j = kt - qb * NJ
                        return j * 128 if j >= 1 else 0

                    for p in range(npair + 2):
                        if p < npair:
                            sc2 = psp.tile([128, 2 * TB], f32, tag="big", bufs=2,
                                           name=f"sc2_{g}_{h}_{p}")
                            for u in range(2):
                                kt = 2 * p + u
                                q0 = q0_of(kt)
                                nc.tensor.matmul(
                                    sc2[:, u * TB + q0:(u + 1) * TB],
                                    kT_sb[:, off + kt * 128: off + (kt + 1) * 128],
                                    qsl[:, q0:],
                                    start=True, stop=True)
                            pt2 = ring.tile([128, 2 * TB], bf16, tag="pt2", bufs=3,
                                            name=f"pt2_{g}_{h}_{p}")
                            if 2 * p - qb * NJ >= 2:
                                # last diagonal pair: exp only the causally
                                # valid tail (mask zeroes the stale rest)
                                for u in range(2):
                                    q0 = q0_of(2 * p + u)
                                    nc.scalar.activation(
                                        pt2[:, u * TB + q0:(u + 1) * TB],
                                        sc2[:, u * TB + q0:(u + 1) * TB], EXP)
                            else:
                                nc.scalar.activation(pt2[:], sc2[:], EXP)
                            jp = p - qb * (NJ // 2)
                            if jp >= 0:
                                # full-width: also zeroes stale data left of q0
                                nc.vector.tensor_mul(pt2[:], pt2[:],
                                                     masks_sb[:, jp, :])
                            pts[p % 4] = pt2
                        pp = p - 2
                        if pp >= 0:
                            ppt = pts[pp % 4]
                            for u in range(2):
                                kt = 2 * pp + u
                                q0 = q0_of(kt)
                                nc.tensor.matmul(
                                    ops[:, q0:TB],
                                    v_sb[:, b * (S // 128) + kt, :],
                                    ppt[:, u * TB + q0:(u + 1) * TB],
                                    start=(kt == 0), stop=(kt == nkt - 1))
                            # denominator partial sums on DVE, emitted after
                            # the AV matmuls so they don't delay the next
                            # pair's mask-mul in the DVE stream
                            if pp == 0:
                                nc.vector.tensor_copy(acc[:], ppt[:])
                            else:
                                nc.vector.tensor_add(acc[:], acc[:], ppt[:])
                    # denominators: reduce acc halves with a ones-matmul
                    sps = psp.tile([1, TB], f32, tag="sum", bufs=2,
                                   name=f"sps{g}_{h}")[:]
                    for u in range(2):
                        nc.tensor.matmul(sps, ones32_sb[:],
                                         acc[:, u * TB:(u + 1) * TB],
                                         start=(u == 0), stop=(u == 1))
                    rec = ring.tile([1, TB], f32, tag="rec", bufs=1,
                                    name=f"rec{g}_{h}")
                    nc.vector.reciprocal_approx_fast(rec[:], sps)
                    rbs = ring.tile([128, TB], f32, tag="rbs", name=f"rbs{g}_{h}")
                    nc.gpsimd.partition_broadcast(rbs[:], rec[:])
                    nc.vector.tensor_mul(atn[:, h, :], ops, rbs[:])

                # B of the next block overlaps the normalize tail of C(g)
                if g + 1 < NG:
                    emit_B(g + 1)

                # ---- D: partial o_proj (1024-wide moving) ----
                for T in range(TB // 128):
                    for n in range(HIDDEN // 1024):
                        ps2 = psp.tile([128, 2 * TB], f32, tag="big", bufs=2,
                                       name=f"od_{g}_{T}_{n}")
                        for half in range(2):
                            for h2 in range(HPC):
                                nc.tensor.matmul(
                                    ps2[:, half * TB:(half + 1) * TB],
                                    atn[:, h2, T * 128:(T + 1) * 128],
                                    wo_sb[:, h2, n * 1024 + half * TB:
                                          n * 1024 + (half + 1) * TB],
                                    start=(h2 == 0), stop=(h2 == HPC - 1))
                        osb = ring.tile([128, 2 * TB], bf16, tag="osb", bufs=2,
                                        name=f"osb{g}_{T}_{n}")
                        nc.vector.tensor_copy(osb[:], ps2[:])
                        nc.sync.dma_start(
                            outp[c0 + T * 128: c0 + (T + 1) * 128,
                                 n * 1024:(n + 1) * 1024], osb[:])

    nc.compile()
    return nc


def make_in_maps(hidden_states, Wq, Wkr, Wdk, Wupk, Wupv, Wo):
    """Host-side sharding + layout prep (bf16). Returns per-core input dicts."""
    import ml_dtypes
    bf16 = ml_dtypes.bfloat16
    scale = 1.0 / np.sqrt(np.float32(HEAD_DIM))

    hidT = np.ascontiguousarray(
        np.asarray(hidden_states, np.float32).reshape(NT, HIDDEN).T).astype(bf16)

    cos_t, sin_t = _rope_tables(S)                     # [128, S] f32
    qcos = (cos_t * scale).astype(bf16)
    qsin = (np.concatenate([-sin_t[0:64], sin_t[64:128]], axis=0) * scale).astype(bf16)
    kcos = np.zeros((128, S), np.float32)
    ksin = np.zeros((128, S), np.float32)
    kcos[64:96] = cos_t[0:32]
    kcos[96:128] = cos_t[64:96]
    ksin[64:96] = -sin_t[0:32]
    ksin[96:128] = sin_t[64:96]
    kcos = kcos.astype(bf16)
    ksin = ksin.astype(bf16)

    k_idx = np.arange(128)[:, None]
    q_idx = np.arange(TB)[None, :]
    m1 = np.stack(
        [(q_idx >= j * 128 + k_idx).astype(np.float32) for j in range(NJ)],
        axis=1)                                        # [128, NJ, TB]
    masks2 = np.concatenate(
        [np.stack([m1[:, 0], m1[:, 2]], axis=1),
         np.stack([m1[:, 1], m1[:, 3]], axis=1)], axis=2).astype(bf16)
    # masks2[:, i, 0:TB] = mask_{2i}, masks2[:, i, TB:2TB] = mask_{2i+1}

    Wq = np.asarray(Wq, np.float32)
    Wkr = np.asarray(Wkr, np.float32)
    Wdk = np.asarray(Wdk, np.float32)
    Wupk = np.asarray(Wupk, np.float32)
    Wupv = np.asarray(Wupv, np.float32)
    Wo = np.asarray(Wo, np.float32)

    in_maps = []
    for c in range(NCORES):
        wq_t = np.ascontiguousarray(Wq[QR * c:QR * (c + 1)].T).astype(bf16)
        wdkkr_t = np.ascontiguousarray(
            np.concatenate([Wdk[LSH * c:LSH * (c + 1)],
                            Wkr[KRR * c:KRR * (c + 1)]], axis=0).T).astype(bf16)
        wupk_t = np.ascontiguousarray(Wupk[KRR * c:KRR * (c + 1)].T).astype(bf16)
        wupv_t = np.ascontiguousarray(
            Wupv[HEAD_DIM * c:HEAD_DIM * (c + 1)].T).astype(bf16)
        wo_t = np.ascontiguousarray(Wo[:, QR * c:QR * (c + 1)].T).astype(bf16)
        in_maps.append({
            "hidT": hidT, "wq_t": wq_t, "wdkkr_t": wdkkr_t,
            "wupk_t": wupk_t, "wupv_t": wupv_t, "wo_t": wo_t,
            "qcos": qcos, "qsin": qsin, "kcos": kcos, "ksin": ksin,
            "masks2": masks2,
        })
    return in_maps


_NC_CACHE = {}


def _get_program(key=0):
    if key not in _NC_CACHE:
        _NC_CACHE[key] = build_program()
    return _NC_CACHE[key]


def finish_output(res):
    out = res.results[0]["out_part"].astype(np.float32)
    for i in range(1, NCORES):
        out = out + res.results[i]["out_part"].astype(np.float32)
    return out.reshape(B, S, HIDDEN).astype(np.float32)


def kernel(hidden_states, Wq, Wkr, Wdk, Wupk, Wupv, Wo):
    from concourse.bass_utils import run_bass_kernel_spmd

    in_maps = make_in_maps(hidden_states, Wq, Wkr, Wdk, Wupk, Wupv, Wo)
    nc = _get_program()
    res = run_bass_kernel_spmd(nc, in_maps, list(range(NCORES)))
    return finish_output(res)


# revision 43
# speedup vs baseline: 1.0307x; 1.0061x over previous
"""MLA (CustomLlamaMLAForInfer) Trainium2 Bass kernel, v3.

Sharding: tensor-parallel over heads across 8 NeuronCores. Core c owns
kv-head c and q-heads [4c, 4c+4). Every core sees the full token stream
(B*S = 4096 tokens). The shared low-rank latent (c_kv, 512 dims) is
*sharded*: core c computes latent dims [64c, 64c+64) for all tokens and
an AllGather rebuilds the full latent on every core. o_proj is
row-sharded; the host sums the 8 partial [4096, 4096] outputs.

All matmuls in bf16 (inputs pre-converted host-side), fp32 PSUM.
One streaming TileContext; PE executes strictly in emission order:

  A(g), g=0..7 (512-token blocks): qT = Wq_shard @ hid.T (rope folded
     in at evict, kept in SBUF), [c_kv shard; k_rope shard] fused
     matmul; c_kv shard -> DRAM -> AllGather (overlapped with later
     A blocks); k_rope roped+scattered into SBUF kT.
  B(g): k_nope/v of the core's kv head from the gathered latent.
  C(g): causal attention for q-block g, 4 q-heads. k-tiles processed
     in PAIRS: two 512-col score matmuls into one 2-bank [128,1024]
     PSUM tile, ONE exp (ScalarE) per pair, paired causal masks,
     v.T@p + ones-matmul denominators, one-pair software pipeline.
  D(g): partial o_proj; PSUM evicted straight to DRAM via DMA (f32).

PSUM (8 banks): big [128,1024] x2 (A q-pairs / C score-pairs),
mid [128,512] x2 (A dk+kr / B knope,v / C out-accum / D o_proj),
sum [1,512] x2 (softmax denominators).
"""

import numpy as np

HIDDEN = 4096
N_HEADS = 32
KV_HEADS = 8
HEAD_DIM = 128
LOW_RANK = 64
TOP_K_ROPE = 32
ROPE_THETA = 10000.0
B, S = 2, 2048
NCORES = 8
HPC = N_HEADS // NCORES          # q heads per core = 4
QR = HPC * HEAD_DIM              # q rows per core = 512
CD = LOW_RANK * KV_HEADS         # latent dim = 512
LSH = CD // NCORES               # latent shard per core = 64
KRR = 2 * TOP_K_ROPE             # rope rows per kv head = 64
NT = B * S                       # total tokens = 4096
TB = 512                         # token block
NG = NT // TB                    # token blocks = 8
HT = HIDDEN // 128               # hidden k-tiles = 32
LT = CD // 128                   # latent k-tiles = 4
NJ = TB // 128                   # diag mask variants = 4


def _rope_tables(seq_len):
    inv = 1.0 / (ROPE_THETA ** (np.arange(0, HEAD_DIM, 2, dtype=np.float32) / HEAD_DIM))
    pos = np.arange(seq_len, dtype=np.float32)
    fr = np.outer(pos, inv)
    emb = np.concatenate([fr, fr], axis=-1)          # [S, 128]
    return (np.cos(emb).T.astype(np.float32),        # [128, S]
            np.sin(emb).T.astype(np.float32))


def build_program(trace_sim=False):
    from concourse import bacc, tile, mybir
    import concourse.bass as bass

    f32 = mybir.dt.float32
    F32R = mybir.dt.float32r
    bf16 = mybir.dt.bfloat16
    MS = bass.MemorySpace
    EXP = mybir.ActivationFunctionType.Exp

    nc = bacc.Bacc("TRN2", target_bir_lowering=False, debug=False,
                   num_devices=NCORES)

    def din(name, shape):
        return nc.dram_tensor(name, shape, bf16, kind="ExternalInput").ap()

    hidT = din("hidT", [HIDDEN, NT])
    wq = din("wq_t", [HIDDEN, QR])
    wdkkr = din("wdkkr_t", [HIDDEN, 128])
    wupk = din("wupk_t", [CD, KRR])
    wupv = din("wupv_t", [CD, HEAD_DIM])
    wo = din("wo_t", [QR, HIDDEN])
    qcos = din("qcos", [128, S])
    qsin = din("qsin", [128, S])
    kcos = din("kcos", [128, S])     # rows 64:128 hold the values
    ksin = din("ksin", [128, S])     # rows 64:128 hold the values
    masks2 = din("masks2", [128, NJ // 2, 2 * TB])
    outp = nc.dram_tensor("out_part", [NT, HIDDEN], bf16,
                          kind="ExternalOutput").ap()

    rg = [list(range(NCORES))]

    with tile.TileContext(nc, trace_sim=trace_sim) as tc:
        with tc.tile_pool(name="pers", bufs=1) as pers, \
             tc.tile_pool(name="ring", bufs=2) as ring, \
             tc.tile_pool(name="dram", bufs=8, space="DRAM") as dram, \
             tc.tile_pool(name="ps", bufs=1, space=MS.PSUM) as psp:

            # ---------------- persistent SBUF ----------------
            # A-phase-critical loads first; wq split so A(0) starts after
            # its first chunk lands.
            wq_sb = pers.tile([128, HT, QR], bf16, tag="wq")
            wqr = wq.rearrange("(t p) w -> p t w", p=128)
            for ch in range(4):
                t0, t1 = ch * (HT // 4), (ch + 1) * (HT // 4)
                nc.sync.dma_start(wq_sb[:, t0:t1, :], wqr[:, t0:t1, :])
            wdkkr_sb = pers.tile([128, HT, 128], bf16, tag="wdkkr")
            nc.sync.dma_start(wdkkr_sb[:], wdkkr.rearrange("(t p) w -> p t w", p=128))
            qcos_sb = pers.tile([128, S], bf16, tag="qcos")
            qsin_sb = pers.tile([128, S], bf16, tag="qsin")
            kcos_sb = pers.tile([128, S], bf16, tag="kcos")
            ksin_sb = pers.tile([128, S], bf16, tag="ksin")
            # BCD-phase tiles (DMAs emitted after phase A so they don't
            # contend with the A-critical hid/wq loads)
            wupk_sb = pers.tile([128, LT, KRR], bf16, tag="wupk")
            wupv_sb = pers.tile([128, LT, HEAD_DIM], bf16, tag="wupv")
            wo_sb = pers.tile([128, HPC, HIDDEN], bf16, tag="wo")
            masks_sb = pers.tile([128, NJ // 2, 2 * TB], bf16, tag="masks")
            ones_f32 = pers.tile([128, 1], f32, tag="ones_f32")
            nc.vector.memset(ones_f32[:], 1.0)
            ones32_sb = pers.tile([128, 1], F32R, tag="ones32")
            nc.scalar.copy(ones32_sb[:], ones_f32[:])

            for z in range(3):
                ptz = ring.tile([128, 2 * TB], bf16, tag="pt2", bufs=3,
                                name=f"ptz{z}")
                nc.vector.memset(ptz[:], 0.0)

            qT_sb = pers.tile([128, HPC, NT], bf16, tag="qT")
            kT_sb = pers.tile([128, NT], bf16, tag="kT")
            v_sb = pers.tile([128, NT // 128, HEAD_DIM], bf16, tag="v")

            gaths = []

            # ================ phase A: projections ================
            for g in range(NG):
                c0, c1 = g * TB, (g + 1) * TB
                p0 = (g % (S // TB)) * TB          # table col (per batch)
                p1 = p0 + TB
                qp2 = [psp.tile([128, 2 * TB], f32, tag="big", bufs=2,
                                name=f"qp2_{g}_{i}") for i in range(2)]
                dkp = psp.tile([128, TB], f32, tag="mid", bufs=2,
                               name=f"dkp{g}")[:]
                for t in range(HT):
                    ht = ring.tile([128, TB], bf16, tag="hid", bufs=10,
                                   name=f"hid{g}_{t}")
                    nc.sync.dma_start(ht[:], hidT[t * 128:(t + 1) * 128, c0:c1])
                    for m in range(HPC):
                        nc.tensor.matmul(
                            qp2[m // 2][:, (m % 2) * TB:(m % 2 + 1) * TB],
                            wq_sb[:, t, m * 128:(m + 1) * 128],
                            ht[:],
                            start=(t == 0), stop=(t == HT - 1))
                    nc.tensor.matmul(dkp, wdkkr_sb[:, t, :], ht[:],
                                     start=(t == 0), stop=(t == HT - 1))
                if g == 0:
                    # rope tables: deferred off the startup critical path,
                    # needed only from the first eviction onward
                    nc.sync.dma_start(qcos_sb[:], qcos)
                    nc.sync.dma_start(qsin_sb[:], qsin)
                    nc.sync.dma_start(kcos_sb[:], kcos)
                    nc.sync.dma_start(ksin_sb[:], ksin)
                # ---- evict q heads with rope ----
                for i in range(2):
                    raw2 = ring.tile([128, 2 * TB], bf16, tag="raw",
                                     name=f"raw{g}_{i}")
                    nc.scalar.copy(raw2[:], qp2[i][:])
                    for mh in range(2):
                        m = 2 * i + mh
                        raw = raw2[:, mh * TB:(mh + 1) * TB]
                        rot = ring.tile([128, TB], bf16, tag="rot",
                                        name=f"rot{g}_{m}")
                        nc.sync.dma_start(rot[0:64, :], raw[64:128, :])
                        nc.sync.dma_start(rot[64:128, :], raw[0:64, :])
                        qsb = ring.tile([128, TB], bf16, tag="qsb",
                                        name=f"qsb{g}_{m}")
                        nc.vector.tensor_mul(qsb[:], raw[:], qcos_sb[:, p0:p1])
                        nc.vector.tensor_mul(rot[:], rot[:], qsin_sb[:, p0:p1])
                        nc.vector.tensor_add(qT_sb[:, m, c0:c1], qsb[:], rot[:])
                # ---- evict latent shard + k rope ----
                dka = ring.tile([128, TB], bf16, tag="dka", name=f"dka{g}")
                nc.scalar.copy(dka[:], dkp)
                ckv_my = dram.tile([LSH, TB], bf16, tag="ckv_my", name=f"ckvmy{g}")
                nc.sync.dma_start(ckv_my[:], dka[0:LSH, :])
                gath = dram.tile([CD, TB], bf16, tag="gath", addr_space="Shared",
                                 name=f"gath{g}")
                nc.gpsimd.collective_compute(
                    "AllGather", mybir.AluOpType.bypass, replica_groups=rg,
                    ins=[ckv_my.opt()], outs=[gath.opt()])
                gaths.append(gath)
                # rope rows live at partitions 64:128
                rotk = ring.tile([128, TB], bf16, tag="rotk", name=f"rotk{g}")
                nc.sync.dma_start(rotk[64:96, :], dka[96:128, :])
                nc.sync.dma_start(rotk[96:128, :], dka[64:96, :])
                ktm = ring.tile([128, TB], bf16, tag="ktm", name=f"ktm{g}")
                nc.vector.tensor_mul(ktm[64:128, :], dka[64:128, :],
                                     kcos_sb[64:128, p0:p1])
                nc.vector.tensor_mul(rotk[64:128, :], rotk[64:128, :],
                                     ksin_sb[64:128, p0:p1])
                nc.vector.tensor_add(ktm[64:128, :], ktm[64:128, :],
                                     rotk[64:128, :])
                nc.sync.dma_start(kT_sb[0:32, c0:c1], ktm[64:96, :])
                nc.sync.dma_start(kT_sb[64:96, c0:c1], ktm[96:128, :])

            # deferred BCD weight loads (complete during remaining A blocks)
            nc.sync.dma_start(wupk_sb[:], wupk.rearrange("(t p) w -> p t w", p=128))
            nc.sync.dma_start(wupv_sb[:], wupv.rearrange("(t p) w -> p t w", p=128))
            nc.sync.dma_start(masks_sb[:], masks2)
            nc.sync.dma_start(wo_sb[:], wo.rearrange("(h p) w -> p h w", p=128))

            # ================ phases B/C/D per block ================
            for g in range(NG):
                c0, c1 = g * TB, (g + 1) * TB
                b, qb = g // (S // TB), g % (S // TB)
                off = b * S
                # ---- B: k_nope + v from gathered latent ----
                cb = ring.tile([128, LT, TB], bf16, tag="cb", name=f"cb{g}")
                nc.sync.dma_start(cb[:], gaths[g].rearrange("(l p) w -> p l w", p=128))
                kcp = psp.tile([KRR, TB], f32, tag="mid", bufs=2,
                               name=f"kcp{g}")[:]
                for lt in range(LT):
                    nc.tensor.matmul(kcp, wupk_sb[:, lt, :], cb[:, lt, :],
                                     start=(lt == 0), stop=(lt == LT - 1))
                kcs = ring.tile([KRR, TB], bf16, tag="kcs", name=f"kcs{g}")
                nc.scalar.copy(kcs[:], kcp)
                nc.sync.dma_start(kT_sb[32:64, c0:c1], kcs[0:32, :])
                nc.sync.dma_start(kT_sb[96:128, c0:c1], kcs[32:64, :])
                for tt in range(TB // 128):
                    vp = psp.tile([128, HEAD_DIM], f32, tag="mid", bufs=2,
                                  name=f"vp{g}_{tt}")[:]
                    for lt in range(LT):
                        nc.tensor.matmul(vp,
                                         cb[:, lt, tt * 128:(tt + 1) * 128],
                                         wupv_sb[:, lt, :],
                                         start=(lt == 0), stop=(lt == LT - 1))
                    nc.scalar.copy(v_sb[:, g * (TB // 128) + tt, :], vp)

                # ---- C: attention for q-block (b, qb), 4 heads ----
                atn = ring.tile([128, HPC, TB], bf16, tag="atn", name=f"atn{g}")
                nkt = (qb + 1) * NJ
                npair = nkt // 2
                for h in range(HPC):
                    qsl = qT_sb[:, h, off + qb * TB: off + (qb + 1) * TB]
                    ops = psp.tile([128, TB], f32, tag="mid", bufs=2,
                                   name=f"ops{g}_{h}")[:]
                    acc = ring.tile([128, 2 * TB], F32R, tag="acc", bufs=2,
                                    name=f"acc{g}_{h}")
                    pts = [None, None, None]
                    # depth-2 software pipeline: scores run 2 pairs ahead
                    # of the AV accumulation.
                    # q0s[kt] = first causally-active q column of k-tile kt;
                    # only the last diagonal pair is restricted (the first
                    # one is nearly full anyway).
                    def q0_of(kt):
                        j = kt - qb * NJ
                        return j * 128 if j >= 1 else 0

                    for p in range(npair + 2):
                        if p < npair:
                            sc2 = psp.tile([128, 2 * TB], f32, tag="big", bufs=2,
                                           name=f"sc2_{g}_{h}_{p}")
                            for u in range(2):
                                kt = 2 * p + u
                                q0 = q0_of(kt)
                                nc.tensor.matmul(
                                    sc2[:, u * TB + q0:(u + 1) * TB],
                                    kT_sb[:, off + kt * 128: off + (kt + 1) * 128],
                                    qsl[:, q0:],
                                    start=True, stop=True)
                            pt2 = ring.tile([128, 2 * TB], bf16, tag="pt2", bufs=3,
                                            name=f"pt2_{g}_{h}_{p}")
                            if 2 * p - qb * NJ >= 2:
                                # last diagonal pair: exp only the causally
                                # valid tail (mask zeroes the stale rest)
                                for u in range(2):
                                    q0 = q0_of(2 * p + u)
                                    nc.scalar.activation(
                                        pt2[:, u * TB + q0:(u + 1) * TB],
                                        sc2[:, u * TB + q0:(u + 1) * TB], EXP)
                            else:
                                nc.scalar.activation(pt2[:], sc2[:], EXP)
                            jp = p - qb * (NJ // 2)
                            if jp >= 0:
                                # full-width: also zeroes stale data left of q0
                                nc.vector.tensor_mul(pt2[:], pt2[:],
                                                     masks_sb[:, jp, :])
                            pts[p % 3] = pt2
                        pp = p - 2
                        if pp >= 0:
                            ppt = pts[pp % 3]
                            for u in range(2):
                                kt = 2 * pp + u
                                q0 = q0_of(kt)
                                nc.tensor.matmul(
                                    ops[:, q0:TB],
                                    v_sb[:, b * (S // 128) + kt, :],
                                    ppt[:, u * TB + q0:(u + 1) * TB],
                                    start=(kt == 0), stop=(kt == nkt - 1))
                            # denominator partial sums on DVE, emitted after
                            # the AV matmuls so they don't delay the next
                            # pair's mask-mul in the DVE stream
                            if pp == 0:
                                nc.vector.tensor_copy(acc[:], ppt[:])
                            else:
                                nc.vector.tensor_add(acc[:], acc[:], ppt[:])
                    # denominators: reduce acc halves with a ones-matmul
                    sps = psp.tile([1, TB], f32, tag="sum", bufs=2,
                                   name=f"sps{g}_{h}")[:]
                    for u in range(2):
                        nc.tensor.matmul(sps, ones32_sb[:],
                                         acc[:, u * TB:(u + 1) * TB],
                                         start=(u == 0), stop=(u == 1))
                    rec = ring.tile([1, TB], f32, tag="rec", bufs=1,
                                    name=f"rec{g}_{h}")
                    nc.vector.reciprocal_approx_fast(rec[:], sps)
                    rbs = ring.tile([128, TB], f32, tag="rbs", name=f"rbs{g}_{h}")
                    nc.gpsimd.partition_broadcast(rbs[:], rec[:])
                    nc.vector.tensor_mul(atn[:, h, :], ops, rbs[:])

                # ---- D: partial o_proj (1024-wide moving) ----
                for T in range(TB // 128):
                    for n in range(HIDDEN // 1024):
                        ps2 = psp.tile([128, 2 * TB], f32, tag="big", bufs=2,
                                       name=f"od_{g}_{T}_{n}")
                        for half in range(2):
                            for h2 in range(HPC):
                                nc.tensor.matmul(
                                    ps2[:, half * TB:(half + 1) * TB],
                                    atn[:, h2, T * 128:(T + 1) * 128],
                                    wo_sb[:, h2, n * 1024 + half * TB:
                                          n * 1024 + (half + 1) * TB],
                                    start=(h2 == 0), stop=(h2 == HPC - 1))
                        osb = ring.tile([128, 2 * TB], bf16, tag="osb", bufs=2,
                                        name=f"osb{g}_{T}_{n}")
                        if n % 2 == 0:
                            nc.scalar.copy(osb[:], ps2[:])
                        else:
                            nc.vector.tensor_copy(osb[:], ps2[:])
                        nc.sync.dma_start(
                            outp[c0 + T * 128: c0 + (T + 1) * 128,
                                 n * 1024:(n + 1) * 1024], osb[:])

    nc.compile()
    return nc


def make_in_maps(hidden_states, Wq, Wkr, Wdk, Wupk, Wupv, Wo):
    """Host-side sharding + layout prep (bf16). Returns per-core input dicts."""
    import ml_dtypes
    bf16 = ml_dtypes.bfloat16
    scale = 1.0 / np.sqrt(np.float32(HEAD_DIM))

    hidT = np.ascontiguousarray(
        np.asarray(hidden_states, np.float32).reshape(NT, HIDDEN).T).astype(bf16)

    cos_t, sin_t = _rope_tables(S)                     # [128, S] f32
    qcos = (cos_t * scale).astype(bf16)
    qsin = (np.concatenate([-sin_t[0:64], sin_t[64:128]], axis=0) * scale).astype(bf16)
    kcos = np.zeros((128, S), np.float32)
    ksin = np.zeros((128, S), np.float32)
    kcos[64:96] = cos_t[0:32]
    kcos[96:128] = cos_t[64:96]
    ksin[64:96] = -sin_t[0:32]
    ksin[96:128] = sin_t[64:96]
    kcos = kcos.astype(bf16)
    ksin = ksin.astype(bf16)

    k_idx = np.arange(128)[:, None]
    q_idx = np.arange(TB)[None, :]
    m1 = np.stack(
        [(q_idx >= j * 128 + k_idx).astype(np.float32) for j in range(NJ)],
        axis=1)                                        # [128, NJ, TB]
    masks2 = np.concatenate(
        [np.stack([m1[:, 0], m1[:, 2]], axis=1),
         np.stack([m1[:, 1], m1[:, 3]], axis=1)], axis=2).astype(bf16)
    # masks2[:, i, 0:TB] = mask_{2i}, masks2[:, i, TB:2TB] = mask_{2i+1}

    Wq = np.asarray(Wq, np.float32)
    Wkr = np.asarray(Wkr, np.float32)
    Wdk = np.asarray(Wdk, np.float32)
    Wupk = np.asarray(Wupk, np.float32)
    Wupv = np.asarray(Wupv, np.float32)
    Wo = np.asarray(Wo, np.float32)

    in_maps = []
    for c in range(NCORES):
        wq_t = np.ascontiguousarray(Wq[QR * c:QR * (c + 1)].T).astype(bf16)
        wdkkr_t = np.ascontiguousarray(
            np.concatenate([Wdk[LSH * c:LSH * (c + 1)],
                            Wkr[KRR * c:KRR * (c + 1)]], axis=0).T).astype(bf16)
        wupk_t = np.ascontiguousarray(Wupk[KRR * c:KRR * (c + 1)].T).astype(bf16)
        wupv_t = np.ascontiguousarray(
            Wupv[HEAD_DIM * c:HEAD_DIM * (c + 1)].T).astype(bf16)
        wo_t = np.ascontiguousarray(Wo[:, QR * c:QR * (c + 1)].T).astype(bf16)
        in_maps.append({
            "hidT": hidT, "wq_t": wq_t, "wdkkr_t": wdkkr_t,
            "wupk_t": wupk_t, "wupv_t": wupv_t, "wo_t": wo_t,
            "qcos": qcos, "qsin": qsin, "kcos": kcos, "ksin": ksin,
            "masks2": masks2,
        })
    return in_maps


_NC_CACHE = {}


def _get_program(key=0):
    if key not in _NC_CACHE:
        _NC_CACHE[key] = build_program()
    return _NC_CACHE[key]


def finish_output(res):
    out = res.results[0]["out_part"].astype(np.float32)
    for i in range(1, NCORES):
        out = out + res.results[i]["out_part"].astype(np.float32)
    return out.reshape(B, S, HIDDEN).astype(np.float32)


def kernel(hidden_states, Wq, Wkr, Wdk, Wupk, Wupv, Wo):
    from concourse.bass_utils import run_bass_kernel_spmd

    in_maps = make_in_maps(hidden_states, Wq, Wkr, Wdk, Wupk, Wupv, Wo)
    nc = _get_program()
    res = run_bass_kernel_spmd(nc, in_maps, list(range(NCORES)))
    return finish_output(res)
